# revision 1
# baseline (speedup 1.0000x reference)
"""Cross-attention kernel for Trainium2 (8 NeuronCores, SPMD).

Reference computation (B=4, Sq=1024, Sk=2048, D=1024, H=16, dh=64):
    q  = x @ Wq + bq                         [B,Sq,D]  -> heads
    kv = ctx @ Wkv + bkv                     [B,Sk,2D] -> k, v heads
    s  = q k^T / sqrt(dh) + mask ; p = softmax(s)
    a  = p v  (merge heads)                  [B,Sq,D]
    out= a @ Wp + bp

Sharding: core c in 0..7 handles batch b=c//2, head-group hg=c%2 (8 of 16
heads).  Column-parallel q/k/v projections, row-parallel c_proj; the two
partial c_proj outputs per batch are summed on the host (cheap, 4MB each).

Device layout (per core) is fully transposed so no on-device transposes are
needed; the host feeds x^T and ctx^T:
    q_t  = Wq_hg^T  @ x^T     [512(heads*dh), Sq]   (4 pair-tiles [128,1024])
    k_t  = Wk_hg^T  @ ctx^T   [512, Sk]             (4 pair-tiles [128,2048])
    v    = ctx @ Wv_hg        [Sk, 512]             (16 tiles [128,512])
    s_t  = k_t-chunk^T q_t  -> [Sk-chunk 128, Sq]   per head (row-tiled pairs)
    p_t  = exp(s_t/8 + mask)  (ACT, f32 psum -> f16 sbuf)
    a_t  = v^T p_t            [dh, Sq] col-tiled head pairs, accum in PSUM
    sums = ones^T @ (sum_c p_t-chunks)  (DVE tree + 1-row matmul)
    a_t /= sums (reciprocal + partition_broadcast + DVE mul)
    outT = wp-chunk^T @ a_t   [D, Sq]  (partial; host adds the two halves,
                                        transposes, adds bp)
All matmuls fp16 operands / fp32 PSUM accumulation.
"""

import sys

import numpy as np

if "/opt/trn_rl_repo" not in sys.path:
    sys.path.insert(0, "/opt/trn_rl_repo")

N_HEAD = 16
B, SQ, SK, D = 4, 1024, 2048, 1024
DH = D // N_HEAD          # 64
HPC = N_HEAD // 2         # 8 heads per core
HCOLS = HPC * DH          # 512 feature cols per core
NCORES = 8

_CACHE = {}


def _build_program(debug=False):
    """Trace the Bass/Tile program once; returns (nc, input names)."""
    import concourse.bass as bass
    import concourse.mybir as mybir
    import concourse.tile as tile
    from concourse import bacc

    f16 = mybir.dt.float16
    f32 = mybir.dt.float32
    AF = mybir.ActivationFunctionType

    nc = bacc.Bacc(
        "TRN2",
        target_bir_lowering=False,
        debug=False,
        enable_asserts=False,
        num_devices=1,
    )

    xT = nc.dram_tensor("xT", [D, SQ], f16, kind="ExternalInput")
    ctxT = nc.dram_tensor("ctxT", [D, SK], f16, kind="ExternalInput")
    wq = nc.dram_tensor("wq", [D, HCOLS], f16, kind="ExternalInput")
    wk = nc.dram_tensor("wk", [D, HCOLS], f16, kind="ExternalInput")
    wv = nc.dram_tensor("wv", [D, HCOLS], f16, kind="ExternalInput")
    wp = nc.dram_tensor("wp", [HCOLS, D], f16, kind="ExternalInput")
    bq_t = nc.dram_tensor("bq_t", [128, 4], f32, kind="ExternalInput")
    bk_t = nc.dram_tensor("bk_t", [128, 4], f32, kind="ExternalInput")
    bv_bc = nc.dram_tensor("bv_bc", [128, HCOLS], f32, kind="ExternalInput")
    mask_t = nc.dram_tensor("mask_t", [128, SK // 128], f32, kind="ExternalInput")
    ones_m = nc.dram_tensor("ones_m", [128, 128], f16, kind="ExternalInput")
    outT = nc.dram_tensor("outT", [D, SQ], f16, kind="ExternalOutput")
    dbg = {}
    if debug:
        dbg["q0"] = nc.dram_tensor("dbg_q0", [128, SQ], f16, kind="ExternalOutput")
        dbg["k0"] = nc.dram_tensor("dbg_k0", [128, SK], f16, kind="ExternalOutput")
        dbg["v0"] = nc.dram_tensor("dbg_v0", [128, HCOLS], f16, kind="ExternalOutput")
        dbg["p00"] = nc.dram_tensor("dbg_p00", [128, 2 * SQ], f16, kind="ExternalOutput")
        dbg["sums0"] = nc.dram_tensor("dbg_sums0", [128, 2 * SQ], f16, kind="ExternalOutput")
        dbg["rbc0"] = nc.dram_tensor("dbg_rbc0", [128, 2 * SQ], f32, kind="ExternalOutput")
        dbg["a0"] = nc.dram_tensor("dbg_a0", [128, SQ], f16, kind="ExternalOutput")

    NKC = SK // 128   # 16 Sk chunks
    from contextlib import ExitStack
    with tile.TileContext(nc) as tc, ExitStack() as stk:
        wpool = stk.enter_context(tc.tile_pool(name="weights", bufs=1))
        apool = stk.enter_context(tc.tile_pool(name="acts", bufs=1))
        psA = stk.enter_context(tc.tile_pool(name="psA", bufs=2, space="PSUM"))
        psB = stk.enter_context(tc.tile_pool(name="psB", bufs=3, space="PSUM"))
        psC = stk.enter_context(tc.tile_pool(name="psC", bufs=1, space="PSUM"))
        proj_stack = ExitStack()
        ppool = proj_stack.enter_context(tc.tile_pool(name="projin", bufs=1))
        if True:
            # ---------------- load inputs ----------------
            xT_sb = [ppool.tile([128, SQ], f16, tag=f"xT{d}", name=f"xT{d}") for d in range(8)]
            cT_sb = [ppool.tile([128, SK], f16, tag=f"cT{d}", name=f"cT{d}") for d in range(8)]
            wq_sb = [ppool.tile([128, HCOLS], f16, tag=f"wq{d}", name=f"wq{d}") for d in range(8)]
            wk_sb = [ppool.tile([128, HCOLS], f16, tag=f"wk{d}", name=f"wk{d}") for d in range(8)]
            wv_sb = [ppool.tile([128, HCOLS], f16, tag=f"wv{d}", name=f"wv{d}") for d in range(8)]
            wp_sb = [wpool.tile([128, D], f16, tag=f"wp{f}", name=f"wpw{f}") for f in range(4)]
            bq_sb = wpool.tile([128, 4], f32, tag="bq", name="bq_sb")
            bk_sb = wpool.tile([128, 4], f32, tag="bk", name="bk_sb")
            bv_sb = wpool.tile([128, HCOLS], f32, tag="bv", name="bv_sb")
            mk_sb = wpool.tile([128, NKC], f32, tag="mk", name="mk_sb")
            om_sb = wpool.tile([128, 128], f16, tag="onesm", name="om_sb")

            for d in range(8):
                nc.sync.dma_start(xT_sb[d][:, :], xT[128 * d:128 * (d + 1), :])
                nc.sync.dma_start(cT_sb[d][:, :], ctxT[128 * d:128 * (d + 1), :])
                nc.sync.dma_start(wq_sb[d][:, :], wq[128 * d:128 * (d + 1), :])
                nc.sync.dma_start(wk_sb[d][:, :], wk[128 * d:128 * (d + 1), :])
                nc.sync.dma_start(wv_sb[d][:, :], wv[128 * d:128 * (d + 1), :])
            for f in range(4):
                nc.sync.dma_start(wp_sb[f][:, :], wp[128 * f:128 * (f + 1), :])
            nc.sync.dma_start(bq_sb[:, :], bq_t[:, :])
            nc.sync.dma_start(bk_sb[:, :], bk_t[:, :])
            nc.sync.dma_start(bv_sb[:, :], bv_bc[:, :])
            nc.sync.dma_start(mk_sb[:, :], mask_t[:, :])
            nc.sync.dma_start(om_sb[:, :], ones_m[:, :])

            # ---------------- projections ----------------
            # q_t pair-tiles [128, SQ]: rows = 2 heads x 64dh
            q_sb = [apool.tile([128, SQ], f16, tag=f"q{p}", name=f"q{p}") for p in range(4)]
            k_sb = [apool.tile([128, SK], f16, tag=f"k{p}", name=f"k{p}") for p in range(4)]
            v_sb = [apool.tile([128, HCOLS], f16, tag=f"v{s}", name=f"v{s}") for s in range(NKC)]

            for p in range(4):
                for n in range(SQ // 512):
                    ps = psC.tile([128, 512], f32, tag="proj", name="proj_ps")
                    for d in range(8):
                        nc.tensor.matmul(
                            ps[:, :],
                            lhsT=wq_sb[d][:, 128 * p:128 * (p + 1)],
                            rhs=xT_sb[d][:, 512 * n:512 * (n + 1)],
                            start=(d == 0), stop=(d == 7),
                        )
                    nc.vector.tensor_scalar_add(
                        q_sb[p][:, 512 * n:512 * (n + 1)], ps[:, :],
                        bq_sb[:, p:p + 1],
                    )
            for p in range(4):
                for n in range(SK // 512):
                    ps = psC.tile([128, 512], f32, tag="proj", name="proj_ps")
                    for d in range(8):
                        nc.tensor.matmul(
                            ps[:, :],
                            lhsT=wk_sb[d][:, 128 * p:128 * (p + 1)],
                            rhs=cT_sb[d][:, 512 * n:512 * (n + 1)],
                            start=(d == 0), stop=(d == 7),
                        )
                    nc.vector.tensor_scalar_add(
                        k_sb[p][:, 512 * n:512 * (n + 1)], ps[:, :],
                        bk_sb[:, p:p + 1],
                    )
            for s in range(NKC):
                ps = psC.tile([128, 512], f32, tag="proj", name="proj_ps")
                for d in range(8):
                    nc.tensor.matmul(
                        ps[:, :],
                        lhsT=cT_sb[d][:, 128 * s:128 * (s + 1)],
                        rhs=wv_sb[d][:, :],
                        start=(d == 0), stop=(d == 7),
                    )
                nc.vector.tensor_add(v_sb[s][:, :], ps[:, :], bv_sb[:, :])

            if debug:
                nc.sync.dma_start(dbg["q0"][:, :], q_sb[0][:, :])
                nc.sync.dma_start(dbg["k0"][:, :], k_sb[0][:, :])
                nc.sync.dma_start(dbg["v0"][:, :], v_sb[0][:, :])
            # ---------------- attention (per head-pair) ----------------
            proj_stack.close()   # free xT/ctxT/wq/wk/wv SBUF for attention
            work = stk.enter_context(tc.tile_pool(name="work", bufs=3))
            norm = stk.enter_context(tc.tile_pool(name="norm", bufs=1))
            a_sb = [apool.tile([128, SQ], f16, tag=f"a{p}", name=f"a{p}") for p in range(4)]
            for p in range(4):
                acc = [psB.tile([128, 512], f32, tag="apsum", name="apsum") for _ in range(2)]
                sums = work.tile([128, 2 * SQ], f16, tag="sums", name="sums", bufs=2)
                for c in range(NKC):
                    pt = work.tile([128, 2 * SQ], f16, tag="p", name="ptile")
                    for a in range(2):
                        sc = psA.tile([128, SQ], f32, tag="scores", name="scores")
                        for n in range(SQ // 512):
                            nc.tensor.matmul(
                                sc[:, 512 * n:512 * (n + 1)],
                                lhsT=k_sb[p][64 * a:64 * (a + 1),
                                             128 * c:128 * (c + 1)],
                                rhs=q_sb[p][64 * a:64 * (a + 1),
                                            512 * n:512 * (n + 1)],
                                tile_position=(64 * a, 0),
                                start=True, stop=True,
                            )
                        # p_t chunk = exp(s/8 + mask)
                        nc.scalar.activation(
                            pt[:, SQ * a:SQ * (a + 1)], sc[:, :],
                            AF.Exp, bias=mk_sb[:, c:c + 1], scale=0.125,
                        )
                    # running column-sum tree on DVE (f16 2x)
                    if c == 0:
                        if debug and p == 0:
                            nc.sync.dma_start(dbg["p00"][:, :], pt[:, :])
                        nc.vector.tensor_copy(sums[:, :], pt[:, :])
                    else:
                        nc.vector.tensor_add(sums[:, :], sums[:, :], pt[:, :])
                    # attn @ v : col-tiled head pair, accumulate over chunks
                    for n in range(SQ // 512):
                        for a in range(2):
                            nc.tensor.matmul(
                                acc[n][64 * a:64 * (a + 1), :],
                                lhsT=v_sb[c][:, 64 * (2 * p + a):
                                             64 * (2 * p + a + 1)],
                                rhs=pt[:, SQ * a + 512 * n:SQ * a + 512 * (n + 1)],
                                tile_position=(0, 64 * a),
                                start=(c == 0), stop=(c == NKC - 1),
                            )
                # --- normalization: R = 1 / colsum(exp) ---
                # ones[128,128]^T @ sums-chunk = column sums broadcast to all
                # 128 partitions in one matmul; then fast reciprocal on DVE.
                r_bc = norm.tile([128, 2 * SQ], f32, tag="rbc", name="rbc")
                for j in range(4):
                    sp = psB.tile([128, 512], f32, tag="apsum", name="sum_ps")
                    nc.tensor.matmul(
                        sp[:, :], lhsT=om_sb[:, :],
                        rhs=sums[:, 512 * j:512 * (j + 1)],
                        start=True, stop=True,
                    )
                    nc.vector.reciprocal_approx_fast(
                        r_bc[:, 512 * j:512 * (j + 1)], sp[:, :])
                if debug and p == 0:
                    nc.sync.dma_start(dbg["rbc0"][:, :], r_bc[:, :])
                for n in range(SQ // 512):
                    for a in range(2):
                        nc.vector.tensor_mul(
                            a_sb[p][64 * a:64 * (a + 1), 512 * n:512 * (n + 1)],
                            acc[n][64 * a:64 * (a + 1), :],
                            r_bc[64 * a:64 * (a + 1),
                                 SQ * a + 512 * n:SQ * a + 512 * (n + 1)],
                        )

            if debug:
                nc.sync.dma_start(dbg["a0"][:, :], a_sb[0][:, :])
            # ---------------- c_proj (row-parallel partial) ----------------
            for dd in range(8):
                ot = work.tile([128, SQ], f16, tag="outT", name="ot", bufs=2)
                for n in range(SQ // 512):
                    ps = psC.tile([128, 512], f32, tag="proj", name="proj_ps")
                    for f in range(4):
                        nc.tensor.matmul(
                            ps[:, :],
                            lhsT=wp_sb[f][:, 128 * dd:128 * (dd + 1)],
                            rhs=a_sb[f][:, 512 * n:512 * (n + 1)],
                            start=(f == 0), stop=(f == 3),
                        )
                    nc.vector.tensor_copy(ot[:, 512 * n:512 * (n + 1)], ps[:, :])
                nc.sync.dma_start(outT[128 * dd:128 * (dd + 1), :], ot[:, :])

    nc.compile()
    return nc


def _shard_inputs(x, ctx, attention_mask, Wq, bq, Wkv, bkv, Wp, bp):
    """Full inputs -> per-core input maps (host numpy)."""
    f16 = np.float16
    in_maps = []
    ones_m = np.ones((128, 128), dtype=f16)
    for c in range(NCORES):
        b, hg = c // 2, c % 2
        lo, hi = hg * HCOLS, (hg + 1) * HCOLS
        m = {
            "xT": np.ascontiguousarray(x[b].T.astype(f16)),
            "ctxT": np.ascontiguousarray(ctx[b].T.astype(f16)),
            "wq": np.ascontiguousarray(Wq[:, lo:hi].astype(f16)),
            "wk": np.ascontiguousarray(Wkv[:, lo:hi].astype(f16)),
            "wv": np.ascontiguousarray(Wkv[:, D + lo:D + hi].astype(f16)),
            "wp": np.ascontiguousarray(Wp[lo:hi, :].astype(f16)),
            "bq_t": np.ascontiguousarray(
                bq[lo:hi].astype(np.float32).reshape(4, 128).T),
            "bk_t": np.ascontiguousarray(
                bkv[lo:hi].astype(np.float32).reshape(4, 128).T),
            "bv_bc": np.ascontiguousarray(np.broadcast_to(
                bkv[D + lo:D + hi].astype(np.float32), (128, HCOLS))),
            "mask_t": np.ascontiguousarray(
                attention_mask[b, 0, 0, :].astype(np.float32)
                .reshape(SK // 128, 128).T),
            "ones_m": ones_m,
        }
        in_maps.append(m)
    return in_maps


def kernel(x, ctx, attention_mask, Wq, bq, Wkv, bkv, Wp, bp, _trace=False,
           _debug=False):
    from concourse.bass_utils import run_bass_kernel_spmd

    x = np.asarray(x); ctx = np.asarray(ctx)
    attention_mask = np.asarray(attention_mask)
    Wq = np.asarray(Wq); bq = np.asarray(bq)
    Wkv = np.asarray(Wkv); bkv = np.asarray(bkv)
    Wp = np.asarray(Wp); bp = np.asarray(bp)

    key = ("nc", _debug)
    if key not in _CACHE:
        _CACHE[key] = _build_program(debug=_debug)
    nc = _CACHE[key]

    in_maps = _shard_inputs(x, ctx, attention_mask, Wq, bq, Wkv, bkv, Wp, bp)
    try:
        res = run_bass_kernel_spmd(
            nc, in_maps, core_ids=list(range(NCORES)), trace=_trace,
        )
    except ModuleNotFoundError:
        # axon NTFF profiling hook unavailable in this container
        res = run_bass_kernel_spmd(
            nc, in_maps, core_ids=list(range(NCORES)), trace=False,
        )
    _CACHE["last_results"] = res

    out = np.empty((B, SQ, D), dtype=np.float32)
    for b in range(B):
        t0 = res.results[2 * b]["outT"].astype(np.float32)
        t1 = res.results[2 * b + 1]["outT"].astype(np.float32)
        out[b] = (t0 + t1).T + bp.astype(np.float32)
    return out



# revision 5
# speedup vs baseline: 3.2293x; 3.2293x over previous
"""Cross-attention kernel for Trainium2 (8 NeuronCores, SPMD).

Reference computation (B=4, Sq=1024, Sk=2048, D=1024, H=16, dh=64):
    q  = x @ Wq + bq                         [B,Sq,D]  -> heads
    kv = ctx @ Wkv + bkv                     [B,Sk,2D] -> k, v heads
    s  = q k^T / sqrt(dh) + mask ; p = softmax(s)
    a  = p v  (merge heads)                  [B,Sq,D]
    out= a @ Wp + bp

Sharding: core c handles batch b=c//2 and query half h=c%2 (rows
[512h, 512h+512) of x[b]) with ALL 16 heads, so each core emits a
complete, disjoint [512, 1024] slice of the output — no host-side
reduction, just a reshape.

The wall-clock bottleneck in this environment is the axon tunnel
(~60 MB/s host<->device both ways, serialized), so the design
minimizes per-call transfer:
  - x and ctx are shipped in natural row-major layout as f16
    (8 MB + 32 MB); transposes happen on-device on the TensorE.
  - weights/biases/identity are shipped once and kept device-resident
    as jax arrays across calls (keyed by a checksum of the weights).
  - the jit(shard_map(bass_exec)) callable is built ONCE and cached;
    (the stock run_bass_kernel_spmd builds a fresh jit per call).
  - PJRT donation zero-buffers are created on-device by a tiny jitted
    zeros-maker instead of being uploaded (saves 8 MB/call).
  - output is a [4096, 1024] f16 shard-concat, fetched once (8 MB).

Device program (per core), all f16 matmul operands / f32 PSUM:
  xT   = transpose(x_half)   via TensorE identity-matmul  [1024, 512]
  cxT  = transpose(ctx)                                    [1024, 2048]
  qT   = Wq^T @ xT + bq       8 tiles [128, 512]   (tile t = 2 heads)
  kT   = Wk^T @ cxT + bk      8 tiles [128, 2048]
  v    = cx @ Wv + bv         16 tiles [128 sk, 1024]
  per head-pair hp: sT = kT-chunk^T qT -> exp(s/8+mask) -> p (f16)
    a^T += v-cols^T p  (PSUM quadrant accumulation over 16 sk chunks)
    colsums via ones^T @ running-f16-sum, reciprocal, broadcast mul
  out  = a @ Wp + bp          natural [512, 1024] via lhsT = a^T tiles
"""

import sys

import numpy as np

if "/opt/trn_rl_repo" not in sys.path:
    sys.path.insert(0, "/opt/trn_rl_repo")

N_HEAD = 16
B, SQ, SK, D = 4, 1024, 2048, 1024
DH = D // N_HEAD          # 64
NCORES = 8
SQH = SQ // 2             # 512 query rows per core
NKC = SK // 128           # 16 sk chunks

_CACHE = {}


def _build_program():
    """Trace the Bass/Tile program once; returns nc."""
    import concourse.bass as bass  # noqa: F401
    import concourse.mybir as mybir
    import concourse.tile as tile
    from concourse import bacc
    from contextlib import ExitStack

    f16 = mybir.dt.float16
    f32 = mybir.dt.float32
    AF = mybir.ActivationFunctionType

    nc = bacc.Bacc(
        "TRN2",
        target_bir_lowering=False,
        debug=False,
        enable_asserts=False,
        num_devices=1,
    )

    x_d = nc.dram_tensor("x", [SQH, D], f16, kind="ExternalInput")
    cx_d = nc.dram_tensor("cx", [SK, D], f16, kind="ExternalInput")
    wq_d = nc.dram_tensor("wq", [D, D], f16, kind="ExternalInput")
    wk_d = nc.dram_tensor("wk", [D, D], f16, kind="ExternalInput")
    wv_d = nc.dram_tensor("wv", [D, D], f16, kind="ExternalInput")
    wp_d = nc.dram_tensor("wp", [D, D], f16, kind="ExternalInput")
    bq_d = nc.dram_tensor("bq_t", [128, 8], f32, kind="ExternalInput")
    bk_d = nc.dram_tensor("bk_t", [128, 8], f32, kind="ExternalInput")
    bv_d = nc.dram_tensor("bv_bc", [128, D], f32, kind="ExternalInput")
    bp_d = nc.dram_tensor("bp_bc", [128, D], f32, kind="ExternalInput")
    mk_d = nc.dram_tensor("mask_t", [128, NKC], f32, kind="ExternalInput")
    id_d = nc.dram_tensor("ident", [128, 128], f16, kind="ExternalInput")
    om_d = nc.dram_tensor("ones_m", [128, 128], f16, kind="ExternalInput")
    out_d = nc.dram_tensor("out", [SQH, D], f16, kind="ExternalOutput")

    with tile.TileContext(nc) as tc, ExitStack() as stk:
        persist = stk.enter_context(tc.tile_pool(name="persist", bufs=1))
        # persistent activations for attention + c_proj
        q_sb = [persist.tile([128, SQH], f16, tag=f"q{t}", name=f"q{t}")
                for t in range(8)]
        k_sb = [persist.tile([128, SK], f16, tag=f"k{t}", name=f"k{t}")
                for t in range(8)]
        v_sb = [persist.tile([128, D], f16, tag=f"v{s}", name=f"v{s}")
                for s in range(NKC)]
        a_sb = [persist.tile([128, SQH], f16, tag=f"a{t}", name=f"a{t}")
                for t in range(8)]
        wp_sb = [persist.tile([128, 512], f16, tag=f"wp{t}{dc}",
                              name=f"wp{t}{dc}")
                 for t in range(8) for dc in range(2)]
        bp_sb = persist.tile([128, D], f32, tag="bp", name="bp_sb")
        mk_sb = persist.tile([128, NKC], f32, tag="mk", name="mk_sb")
        om_sb = persist.tile([128, 128], f16, tag="onesm", name="om_sb")
        for t in range(8):
            for dc in range(2):
                nc.sync.dma_start(
                    wp_sb[2 * t + dc][:, :],
                    wp_d[128 * t:128 * (t + 1), 512 * dc:512 * (dc + 1)])
        nc.sync.dma_start(bp_sb[:, :], bp_d[:, :])
        nc.sync.dma_start(mk_sb[:, :], mk_d[:, :])
        nc.sync.dma_start(om_sb[:, :], om_d[:, :])

        # ---------------- Phase A: transposes + projections ----------------
        stageA = ExitStack()
        mpool = stageA.enter_context(tc.tile_pool(name="midA", bufs=1))
        psT = stageA.enter_context(
            tc.tile_pool(name="psT", bufs=2, space="PSUM"))
        psP = stageA.enter_context(
            tc.tile_pool(name="psP", bufs=2, space="PSUM"))

        id_sb = mpool.tile([128, 128], f16, tag="ident", name="id_sb")
        bq_sb = mpool.tile([128, 8], f32, tag="bq", name="bq_sb")
        bk_sb = mpool.tile([128, 8], f32, tag="bk", name="bk_sb")
        bv_sb = mpool.tile([128, D], f32, tag="bv", name="bv_sb")
        nc.sync.dma_start(id_sb[:, :], id_d[:, :])
        nc.sync.dma_start(bq_sb[:, :], bq_d[:, :])
        nc.sync.dma_start(bk_sb[:, :], bk_d[:, :])
        nc.sync.dma_start(bv_sb[:, :], bv_d[:, :])

        xT_sb = [mpool.tile([128, SQH], f16, tag=f"xT{j}", name=f"xT{j}")
                 for j in range(8)]
        cxT_sb = [mpool.tile([128, SK], f16, tag=f"cxT{j}", name=f"cxT{j}")
                  for j in range(8)]

        # sub-stage A1: raw x/cx tiles, freed after the transposes
        stageA1 = ExitStack()
        rpool = stageA1.enter_context(tc.tile_pool(name="rawA", bufs=1))
        x_sb = [rpool.tile([128, D], f16, tag=f"x{i}", name=f"x{i}")
                for i in range(4)]
        cx_sb = [rpool.tile([128, D], f16, tag=f"cx{i}", name=f"cx{i}")
                 for i in range(16)]
        for i in range(4):
            nc.sync.dma_start(x_sb[i][:, :], x_d[128 * i:128 * (i + 1), :])
        for i in range(16):
            nc.sync.dma_start(cx_sb[i][:, :], cx_d[128 * i:128 * (i + 1), :])
        for j in range(8):
            for i in range(4):
                pt = psT.tile([128, 128], f16, tag="tp", name="tp")
                nc.tensor.transpose(
                    pt[:, :], x_sb[i][:, 128 * j:128 * (j + 1)], id_sb[:, :])
                nc.vector.tensor_copy(
                    xT_sb[j][:, 128 * i:128 * (i + 1)], pt[:, :])
        for j in range(8):
            for i in range(16):
                pt = psT.tile([128, 128], f16, tag="tp", name="tp")
                nc.tensor.transpose(
                    pt[:, :], cx_sb[i][:, 128 * j:128 * (j + 1)], id_sb[:, :])
                nc.vector.tensor_copy(
                    cxT_sb[j][:, 128 * i:128 * (i + 1)], pt[:, :])
        stageA1.close()

        # sub-stage A2: projection weights
        stageA2 = ExitStack()
        wpool = stageA2.enter_context(tc.tile_pool(name="wA", bufs=1))
        wq_sb = [wpool.tile([128, D], f16, tag=f"wq{d}", name=f"wq{d}")
                 for d in range(8)]
        wk_sb = [wpool.tile([128, D], f16, tag=f"wk{d}", name=f"wk{d}")
                 for d in range(8)]
        wv_sb = [wpool.tile([128, D], f16, tag=f"wv{d}", name=f"wv{d}")
                 for d in range(8)]
        for d in range(8):
            nc.sync.dma_start(wq_sb[d][:, :], wq_d[128 * d:128 * (d + 1), :])
            nc.sync.dma_start(wk_sb[d][:, :], wk_d[128 * d:128 * (d + 1), :])
            nc.sync.dma_start(wv_sb[d][:, :], wv_d[128 * d:128 * (d + 1), :])

        # qT tiles [128, 512]: tile t = features 128t..128t+127
        for t in range(8):
            ps = psP.tile([128, 512], f32, tag="proj", name="proj_ps")
            for d in range(8):
                nc.tensor.matmul(
                    ps[:, :],
                    lhsT=wq_sb[d][:, 128 * t:128 * (t + 1)],
                    rhs=xT_sb[d][:, :],
                    start=(d == 0), stop=(d == 7),
                )
            nc.vector.tensor_scalar_add(
                q_sb[t][:, :], ps[:, :], bq_sb[:, t:t + 1])
        # kT tiles [128, 2048] in 4 sk chunks of 512
        for t in range(8):
            for n in range(4):
                ps = psP.tile([128, 512], f32, tag="proj", name="proj_ps")
                for d in range(8):
                    nc.tensor.matmul(
                        ps[:, :],
                        lhsT=wk_sb[d][:, 128 * t:128 * (t + 1)],
                        rhs=cxT_sb[d][:, 512 * n:512 * (n + 1)],
                        start=(d == 0), stop=(d == 7),
                    )
                nc.vector.tensor_scalar_add(
                    k_sb[t][:, 512 * n:512 * (n + 1)], ps[:, :],
                    bk_sb[:, t:t + 1])
        # v tiles [128 sk, 1024] in 2 d chunks of 512
        for s in range(NKC):
            for dc in range(2):
                ps = psP.tile([128, 512], f32, tag="proj", name="proj_ps")
                for d in range(8):
                    nc.tensor.matmul(
                        ps[:, :],
                        lhsT=cxT_sb[d][:, 128 * s:128 * (s + 1)],
                        rhs=wv_sb[d][:, 512 * dc:512 * (dc + 1)],
                        start=(d == 0), stop=(d == 7),
                    )
                nc.vector.tensor_add(
                    v_sb[s][:, 512 * dc:512 * (dc + 1)], ps[:, :],
                    bv_sb[:, 512 * dc:512 * (dc + 1)])

        stageA2.close()
        stageA.close()   # free xT/cxT/weights SBUF + psum pools

        # ---------------- Phase B: attention per head-pair ----------------
        stageB = ExitStack()
        work = stageB.enter_context(tc.tile_pool(name="work", bufs=3))
        norm = stageB.enter_context(tc.tile_pool(name="norm", bufs=1))
        psS = stageB.enter_context(
            tc.tile_pool(name="psS", bufs=3, space="PSUM"))
        psAcc = stageB.enter_context(
            tc.tile_pool(name="psAcc", bufs=2, space="PSUM"))
        psSum = stageB.enter_context(
            tc.tile_pool(name="psSum", bufs=2, space="PSUM"))

        for hp in range(8):
            acc = psAcc.tile([128, SQH], f32, tag="apsum", name="apsum")
            sums = work.tile([128, 2 * SQH], f16, tag="sums", name="sums",
                             bufs=2)
            for c in range(NKC):
                pt = work.tile([128, 2 * SQH], f16, tag="p", name="ptile")
                for a in range(2):
                    sc = psS.tile([128, SQH], f32, tag="scores",
                                  name="scores")
                    nc.tensor.matmul(
                        sc[:, :],
                        lhsT=k_sb[hp][64 * a:64 * (a + 1),
                                      128 * c:128 * (c + 1)],
                        rhs=q_sb[hp][64 * a:64 * (a + 1), :],
                        tile_position=(64 * a, 0),
                        start=True, stop=True,
                    )
                    nc.scalar.activation(
                        pt[:, SQH * a:SQH * (a + 1)], sc[:, :],
                        AF.Exp, bias=mk_sb[:, c:c + 1], scale=0.125,
                    )
                if c == 0:
                    nc.vector.tensor_copy(sums[:, :], pt[:, :])
                else:
                    nc.vector.tensor_add(sums[:, :], sums[:, :], pt[:, :])
                for a in range(2):
                    nc.tensor.matmul(
                        acc[64 * a:64 * (a + 1), :],
                        lhsT=v_sb[c][:, 64 * (2 * hp + a):
                                     64 * (2 * hp + a + 1)],
                        rhs=pt[:, SQH * a:SQH * (a + 1)],
                        tile_position=(0, 64 * a),
                        start=(c == 0), stop=(c == NKC - 1),
                    )
            # normalization: R = 1 / colsum(exp) broadcast to partitions
            r_bc = norm.tile([128, 2 * SQH], f32, tag="rbc", name="rbc")
            for j in range(2):
                sp = psSum.tile([128, 512], f32, tag="sum_ps", name="sum_ps")
                nc.tensor.matmul(
                    sp[:, :], lhsT=om_sb[:, :],
                    rhs=sums[:, 512 * j:512 * (j + 1)],
                    start=True, stop=True,
                )
                nc.vector.reciprocal_approx_fast(
                    r_bc[:, 512 * j:512 * (j + 1)], sp[:, :])
            for a in range(2):
                nc.vector.tensor_mul(
                    a_sb[hp][64 * a:64 * (a + 1), :],
                    acc[64 * a:64 * (a + 1), :],
                    r_bc[64 * a:64 * (a + 1), SQH * a:SQH * (a + 1)],
                )
        stageB.close()

        # ---------------- Phase C: c_proj, natural [sq, d] layout ---------
        stageC = ExitStack()
        cpool = stageC.enter_context(tc.tile_pool(name="stageC", bufs=2))
        psO = stageC.enter_context(
            tc.tile_pool(name="psO", bufs=2, space="PSUM"))
        for sqc in range(4):
            ot = cpool.tile([128, D], f16, tag="outT", name="ot")
            for dc in range(2):
                ps = psO.tile([128, 512], f32, tag="ops", name="ops")
                for hp in range(8):
                    nc.tensor.matmul(
                        ps[:, :],
                        lhsT=a_sb[hp][:, 128 * sqc:128 * (sqc + 1)],
                        rhs=wp_sb[2 * hp + dc][:, :],
                        start=(hp == 0), stop=(hp == 7),
                    )
                nc.vector.tensor_add(
                    ot[:, 512 * dc:512 * (dc + 1)], ps[:, :],
                    bp_sb[:, 512 * dc:512 * (dc + 1)])
            nc.sync.dma_start(out_d[128 * sqc:128 * (sqc + 1), :], ot[:, :])
        stageC.close()

    nc.compile()
    return nc


class _Runner:
    """Cached jit(shard_map(bass_exec)) with device-resident weights."""

    def __init__(self, nc):
        import jax
        import concourse.mybir as mybir
        from concourse import bass2jax
        from jax.sharding import Mesh, NamedSharding, PartitionSpec
        from jax.experimental.shard_map import shard_map

        bass2jax.install_neuronx_cc_hook()
        self.jax = jax
        self.nc = nc

        partition_name = (nc.partition_id_tensor.name
                          if nc.partition_id_tensor else None)
        in_names, out_names, out_avals = [], [], []
        for alloc in nc.m.functions[0].allocations:
            if not isinstance(alloc, mybir.MemoryLocationSet):
                continue
            name = alloc.memorylocations[0].name
            if alloc.kind == "ExternalInput":
                if name != partition_name:
                    in_names.append(name)
            elif alloc.kind == "ExternalOutput":
                out_names.append(name)
                out_avals.append(jax.core.ShapedArray(
                    tuple(alloc.tensor_shape), mybir.dt.np(alloc.dtype)))
        self.in_names = in_names
        self.out_names = out_names
        self.out_avals = out_avals
        n_params = len(in_names)
        bind_names = tuple(in_names + out_names +
                           ([partition_name] if partition_name else []))

        devices = jax.devices()[:NCORES]
        self.mesh = Mesh(np.asarray(devices), ("core",))
        self.shard = NamedSharding(self.mesh, PartitionSpec("core"))

        def _body(*args):
            operands = list(args)
            if partition_name is not None:
                operands.append(bass2jax.partition_id_tensor())
            outs = bass2jax._bass_exec_p.bind(
                *operands,
                out_avals=tuple(out_avals),
                in_names=bind_names,
                out_names=tuple(out_names),
                lowering_input_output_aliases=(),
                sim_require_finite=True,
                sim_require_nnan=True,
                nc=nc,
            )
            return tuple(outs)

        donate = tuple(range(n_params, n_params + len(out_names)))
        self.sharded = jax.jit(
            shard_map(_body, mesh=self.mesh,
                      in_specs=(PartitionSpec("core"),) * (n_params +
                                                           len(out_names)),
                      out_specs=(PartitionSpec("core"),) * len(out_names),
                      check_rep=False),
            donate_argnums=donate, keep_unused=True)

        import jax.numpy as jnp
        zero_shards = [NamedSharding(self.mesh, PartitionSpec("core"))
                       for _ in out_avals]

        def _mk_zeros():
            return tuple(
                jnp.zeros((NCORES * a.shape[0],) + tuple(a.shape[1:]),
                          a.dtype)
                for a in out_avals)

        self.mk_zeros = jax.jit(_mk_zeros, out_shardings=tuple(zero_shards))

    def put(self, arr):
        """Host concat array -> device-resident sharded jax array."""
        return self.jax.device_put(arr, self.shard)

    def run(self, operands):
        """operands: dict name -> array (jax device or numpy concat)."""
        zeros = self.mk_zeros()
        args = [operands[n] for n in self.in_names] + list(zeros)
        outs = self.sharded(*args)
        return outs


def _checksum(arr):
    a = np.ascontiguousarray(arr)
    return (a.shape, a.dtype.str,
            int(a.view(np.uint8).reshape(-1)[::4097].sum()),
            int(a.view(np.uint8).reshape(-1)[-65536::257].sum()))


def _prep_weights(runner, Wq, bq, Wkv, bkv, Wp, bp):
    """Device-resident per-core weight shards (identical on all cores)."""
    f16, f32 = np.float16, np.float32

    def rep(a):   # replicate per core along axis0 for shard_map concat
        return np.ascontiguousarray(
            np.broadcast_to(a, (NCORES,) + a.shape)).reshape(
                (NCORES * a.shape[0],) + a.shape[1:])

    wq = Wq.astype(f16)
    wk = Wkv[:, :D].astype(f16)
    wv = Wkv[:, D:].astype(f16)
    wp = Wp.astype(f16)
    bq_t = np.ascontiguousarray(bq.astype(f32).reshape(8, 128).T)
    bk_t = np.ascontiguousarray(bkv[:D].astype(f32).reshape(8, 128).T)
    bv_bc = np.ascontiguousarray(
        np.broadcast_to(bkv[D:].astype(f32), (128, D)))
    bp_bc = np.ascontiguousarray(
        np.broadcast_to(bp.astype(f32), (128, D)))
    ident = np.eye(128, dtype=f16)
    ones_m = np.ones((128, 128), dtype=f16)
    host = {
        "wq": rep(wq), "wk": rep(wk), "wv": rep(wv), "wp": rep(wp),
        "bq_t": rep(bq_t), "bk_t": rep(bk_t), "bv_bc": rep(bv_bc),
        "bp_bc": rep(bp_bc), "ident": rep(ident), "ones_m": rep(ones_m),
    }
    return {k: runner.put(v) for k, v in host.items()}


def _prep_acts(x, ctx, attention_mask):
    """Per-call activations: natural layout f16, zero-copy where possible."""
    f16, f32 = np.float16, np.float32
    x16 = np.asarray(x, dtype=f16)                       # [4,1024,1024]
    cx16 = np.asarray(ctx, dtype=f16)                    # [4,2048,1024]
    concat_x = x16.reshape(NCORES * SQH, D)              # zero-copy
    concat_cx = cx16[[0, 0, 1, 1, 2, 2, 3, 3]].reshape(NCORES * SK, D)
    m = np.asarray(attention_mask, dtype=f32).reshape(B, SK)
    mask_t = np.ascontiguousarray(
        m.reshape(B, NKC, 128).transpose(0, 2, 1))       # [4,128,16]
    concat_mask = np.ascontiguousarray(
        mask_t[[0, 0, 1, 1, 2, 2, 3, 3]]).reshape(NCORES * 128, NKC)
    return {"x": concat_x, "cx": concat_cx, "mask_t": concat_mask}


def kernel(x, ctx, attention_mask, Wq, bq, Wkv, bkv, Wp, bp, **_ignored):
    x = np.asarray(x); ctx = np.asarray(ctx)
    attention_mask = np.asarray(attention_mask)
    Wq = np.asarray(Wq); bq = np.asarray(bq)
    Wkv = np.asarray(Wkv); bkv = np.asarray(bkv)
    Wp = np.asarray(Wp); bp = np.asarray(bp)

    if "nc" not in _CACHE:
        _CACHE["nc"] = _build_program()
    nc = _CACHE["nc"]
    if "runner" not in _CACHE:
        _CACHE["runner"] = _Runner(nc)
    runner = _CACHE["runner"]

    wkey = tuple(_checksum(a) for a in (Wq, bq, Wkv, bkv, Wp, bp))
    if _CACHE.get("wkey") != wkey:
        _CACHE["weights"] = _prep_weights(runner, Wq, bq, Wkv, bkv, Wp, bp)
        _CACHE["wkey"] = wkey

    operands = dict(_CACHE["weights"])
    operands.update(_prep_acts(x, ctx, attention_mask))
    outs = runner.run(operands)
    out16 = np.asarray(outs[0])                          # [8*512, 1024] f16
    return out16.reshape(B, SQ, D).astype(np.float32)


# revision 6
# speedup vs baseline: 4.7588x; 1.4736x over previous
"""Cross-attention kernel for Trainium2 (8 NeuronCores, SPMD).

Reference computation (B=4, Sq=1024, Sk=2048, D=1024, H=16, dh=64):
    q  = x @ Wq + bq                         [B,Sq,D]  -> heads
    kv = ctx @ Wkv + bkv                     [B,Sk,2D] -> k, v heads
    s  = q k^T / sqrt(dh) + mask ; p = softmax(s)
    a  = p v  (merge heads)                  [B,Sq,D]
    out= a @ Wp + bp

Sharding: core c handles batch b=c//2 and query half h=c%2 (rows
[512h, 512h+512) of x[b]) with ALL 16 heads, so each core emits a
complete, disjoint [512, 1024] slice of the output — no host-side
reduction, just a reshape.  Each core uploads only its own ctx HALF
(rows [1024h, 1024h+1024) of ctx[b]); the k/v halves are exchanged
between the two cores of a batch with an on-device pairwise HBM
AllGather (replica groups {0,1},{2,3},{4,5},{6,7}).

The wall-clock bottleneck in this environment is the axon tunnel
(~60 MB/s host<->device both ways, serialized), so the design
minimizes per-call transfer:
  - x and ctx are shipped in natural row-major layout as f16
    (8 MB + 32 MB); transposes happen on-device on the TensorE.
  - weights/biases/identity are shipped once and kept device-resident
    as jax arrays across calls (keyed by a checksum of the weights).
  - the jit(shard_map(bass_exec)) callable is built ONCE and cached;
    (the stock run_bass_kernel_spmd builds a fresh jit per call).
  - PJRT donation zero-buffers are created on-device by a tiny jitted
    zeros-maker instead of being uploaded (saves 8 MB/call).
  - output is a [4096, 1024] f16 shard-concat, fetched once (8 MB).

Device program (per core), all f16 matmul operands / f32 PSUM:
  xT   = transpose(x_half)   via TensorE identity-matmul  [1024, 512]
  cxT  = transpose(ctx)                                    [1024, 2048]
  qT   = Wq^T @ xT + bq       8 tiles [128, 512]   (tile t = 2 heads)
  kT   = Wk^T @ cxT + bk      8 tiles [128, 2048]
  v    = cx @ Wv + bv         16 tiles [128 sk, 1024]
  per head-pair hp: sT = kT-chunk^T qT -> exp(s/8+mask) -> p (f16)
    a^T += v-cols^T p  (PSUM quadrant accumulation over 16 sk chunks)
    colsums via ones^T @ running-f16-sum, reciprocal, broadcast mul
  out  = a @ Wp + bp          natural [512, 1024] via lhsT = a^T tiles
"""

import sys

import numpy as np

if "/opt/trn_rl_repo" not in sys.path:
    sys.path.insert(0, "/opt/trn_rl_repo")

N_HEAD = 16
B, SQ, SK, D = 4, 1024, 2048, 1024
DH = D // N_HEAD          # 64
NCORES = 8
SQH = SQ // 2             # 512 query rows per core
NKC = SK // 128           # 16 sk chunks

_CACHE = {}


def _build_program():
    """Trace the Bass/Tile program once; returns nc."""
    import concourse.bass as bass  # noqa: F401
    import concourse.mybir as mybir
    import concourse.tile as tile
    from concourse import bacc
    from contextlib import ExitStack

    f16 = mybir.dt.float16
    f32 = mybir.dt.float32
    AF = mybir.ActivationFunctionType

    nc = bacc.Bacc(
        "TRN2",
        target_bir_lowering=False,
        debug=False,
        enable_asserts=False,
        num_devices=NCORES,
    )

    SKH = SK // 2
    x_d = nc.dram_tensor("x", [SQH, D], f16, kind="ExternalInput")
    cx_d = nc.dram_tensor("cx", [SKH, D], f16, kind="ExternalInput")
    wq_d = nc.dram_tensor("wq", [D, D], f16, kind="ExternalInput")
    wk_d = nc.dram_tensor("wk", [D, D], f16, kind="ExternalInput")
    wv_d = nc.dram_tensor("wv", [D, D], f16, kind="ExternalInput")
    wp_d = nc.dram_tensor("wp", [D, D], f16, kind="ExternalInput")
    bq_d = nc.dram_tensor("bq_t", [128, 8], f32, kind="ExternalInput")
    bk_d = nc.dram_tensor("bk_t", [128, 8], f32, kind="ExternalInput")
    bv_d = nc.dram_tensor("bv_bc", [128, D], f32, kind="ExternalInput")
    bp_d = nc.dram_tensor("bp_bc", [128, D], f32, kind="ExternalInput")
    mk_d = nc.dram_tensor("mask_t", [128, NKC], f32, kind="ExternalInput")
    id_d = nc.dram_tensor("ident", [128, 128], f16, kind="ExternalInput")
    om_d = nc.dram_tensor("ones_m", [128, 128], f16, kind="ExternalInput")
    out_d = nc.dram_tensor("out", [SQH, D], f16, kind="ExternalOutput")

    with tile.TileContext(nc) as tc, ExitStack() as stk:
        persist = stk.enter_context(tc.tile_pool(name="persist", bufs=1))
        # persistent activations for attention + c_proj
        q_sb = [persist.tile([128, SQH], f16, tag=f"q{t}", name=f"q{t}")
                for t in range(8)]
        k_sb = [persist.tile([128, SK], f16, tag=f"k{t}", name=f"k{t}")
                for t in range(8)]
        v_sb = [persist.tile([128, D], f16, tag=f"v{s}", name=f"v{s}")
                for s in range(NKC)]
        a_sb = [persist.tile([128, SQH], f16, tag=f"a{t}", name=f"a{t}")
                for t in range(8)]
        wp_sb = [persist.tile([128, 512], f16, tag=f"wp{t}{dc}",
                              name=f"wp{t}{dc}")
                 for t in range(8) for dc in range(2)]
        bp_sb = persist.tile([128, D], f32, tag="bp", name="bp_sb")
        mk_sb = persist.tile([128, NKC], f32, tag="mk", name="mk_sb")
        om_sb = persist.tile([128, 128], f16, tag="onesm", name="om_sb")
        for t in range(8):
            for dc in range(2):
                nc.sync.dma_start(
                    wp_sb[2 * t + dc][:, :],
                    wp_d[128 * t:128 * (t + 1), 512 * dc:512 * (dc + 1)])
        nc.sync.dma_start(bp_sb[:, :], bp_d[:, :])
        nc.sync.dma_start(mk_sb[:, :], mk_d[:, :])
        nc.sync.dma_start(om_sb[:, :], om_d[:, :])

        # ---------------- Phase A: transposes + projections ----------------
        stageA = ExitStack()
        mpool = stageA.enter_context(tc.tile_pool(name="midA", bufs=1))
        psT = stageA.enter_context(
            tc.tile_pool(name="psT", bufs=2, space="PSUM"))
        psP = stageA.enter_context(
            tc.tile_pool(name="psP", bufs=2, space="PSUM"))

        id_sb = mpool.tile([128, 128], f16, tag="ident", name="id_sb")
        bq_sb = mpool.tile([128, 8], f32, tag="bq", name="bq_sb")
        bk_sb = mpool.tile([128, 8], f32, tag="bk", name="bk_sb")
        bv_sb = mpool.tile([128, D], f32, tag="bv", name="bv_sb")
        nc.sync.dma_start(id_sb[:, :], id_d[:, :])
        nc.sync.dma_start(bq_sb[:, :], bq_d[:, :])
        nc.sync.dma_start(bk_sb[:, :], bk_d[:, :])
        nc.sync.dma_start(bv_sb[:, :], bv_d[:, :])

        xT_sb = [mpool.tile([128, SQH], f16, tag=f"xT{j}", name=f"xT{j}")
                 for j in range(8)]
        cxT_sb = [mpool.tile([128, SKH], f16, tag=f"cxT{j}", name=f"cxT{j}")
                  for j in range(8)]

        # DRAM bounce buffers for the pairwise kv AllGather
        dpool = stageA.enter_context(
            tc.tile_pool(name="dramA", bufs=1, space="DRAM"))
        kv_in = dpool.tile([2 * SKH, D], f16, tag="kvin", name="kv_in")
        kv_out = dpool.tile([4 * SKH, D], f16, tag="kvout", name="kv_out")

        # sub-stage A1: raw x/cx tiles, freed after the transposes
        stageA1 = ExitStack()
        rpool = stageA1.enter_context(tc.tile_pool(name="rawA", bufs=1))
        x_sb = [rpool.tile([128, D], f16, tag=f"x{i}", name=f"x{i}")
                for i in range(4)]
        cx_sb = [rpool.tile([128, D], f16, tag=f"cx{i}", name=f"cx{i}")
                 for i in range(8)]
        for i in range(4):
            nc.sync.dma_start(x_sb[i][:, :], x_d[128 * i:128 * (i + 1), :])
        for i in range(8):
            nc.sync.dma_start(cx_sb[i][:, :], cx_d[128 * i:128 * (i + 1), :])
        for j in range(8):
            for i in range(4):
                pt = psT.tile([128, 128], f16, tag="tp", name="tp")
                nc.tensor.transpose(
                    pt[:, :], x_sb[i][:, 128 * j:128 * (j + 1)], id_sb[:, :])
                nc.vector.tensor_copy(
                    xT_sb[j][:, 128 * i:128 * (i + 1)], pt[:, :])
        for j in range(8):
            for i in range(8):
                pt = psT.tile([128, 128], f16, tag="tp", name="tp")
                nc.tensor.transpose(
                    pt[:, :], cx_sb[i][:, 128 * j:128 * (j + 1)], id_sb[:, :])
                nc.vector.tensor_copy(
                    cxT_sb[j][:, 128 * i:128 * (i + 1)], pt[:, :])
        stageA1.close()

        # sub-stage A2: projection weights
        stageA2 = ExitStack()
        wpool = stageA2.enter_context(tc.tile_pool(name="wA", bufs=1))
        wq_sb = [wpool.tile([128, D], f16, tag=f"wq{d}", name=f"wq{d}")
                 for d in range(8)]
        wk_sb = [wpool.tile([128, D], f16, tag=f"wk{d}", name=f"wk{d}")
                 for d in range(8)]
        wv_sb = [wpool.tile([128, D], f16, tag=f"wv{d}", name=f"wv{d}")
                 for d in range(8)]
        for d in range(8):
            nc.sync.dma_start(wq_sb[d][:, :], wq_d[128 * d:128 * (d + 1), :])
            nc.sync.dma_start(wk_sb[d][:, :], wk_d[128 * d:128 * (d + 1), :])
            nc.sync.dma_start(wv_sb[d][:, :], wv_d[128 * d:128 * (d + 1), :])

        # qT tiles [128, 512]: tile t = features 128t..128t+127
        for t in range(8):
            ps = psP.tile([128, 512], f32, tag="proj", name="proj_ps")
            for d in range(8):
                nc.tensor.matmul(
                    ps[:, :],
                    lhsT=wq_sb[d][:, 128 * t:128 * (t + 1)],
                    rhs=xT_sb[d][:, :],
                    start=(d == 0), stop=(d == 7),
                )
            nc.vector.tensor_scalar_add(
                q_sb[t][:, :], ps[:, :], bq_sb[:, t:t + 1])
        # own-half kT tiles [128, 1024] -> DRAM bounce rows [128t, :]
        hpool = stageA2.enter_context(tc.tile_pool(name="halves", bufs=2))
        for t in range(8):
            kh = hpool.tile([128, SKH], f16, tag="kh", name="kh")
            for n in range(2):
                ps = psP.tile([128, 512], f32, tag="proj", name="proj_ps")
                for d in range(8):
                    nc.tensor.matmul(
                        ps[:, :],
                        lhsT=wk_sb[d][:, 128 * t:128 * (t + 1)],
                        rhs=cxT_sb[d][:, 512 * n:512 * (n + 1)],
                        start=(d == 0), stop=(d == 7),
                    )
                nc.vector.tensor_scalar_add(
                    kh[:, 512 * n:512 * (n + 1)], ps[:, :],
                    bk_sb[:, t:t + 1])
            nc.sync.dma_start(kv_in[128 * t:128 * (t + 1), :], kh[:, :])
        # own-half v tiles [128 skh, 1024] -> bounce rows [SKH + 128s, :]
        for s in range(8):
            vh = hpool.tile([128, D], f16, tag="vh", name="vh")
            for dc in range(2):
                ps = psP.tile([128, 512], f32, tag="proj", name="proj_ps")
                for d in range(8):
                    nc.tensor.matmul(
                        ps[:, :],
                        lhsT=cxT_sb[d][:, 128 * s:128 * (s + 1)],
                        rhs=wv_sb[d][:, 512 * dc:512 * (dc + 1)],
                        start=(d == 0), stop=(d == 7),
                    )
                nc.vector.tensor_add(
                    vh[:, 512 * dc:512 * (dc + 1)], ps[:, :],
                    bv_sb[:, 512 * dc:512 * (dc + 1)])
            nc.sync.dma_start(kv_in[SKH + 128 * s:SKH + 128 * (s + 1), :],
                              vh[:, :])

        # pairwise exchange: cores (2b, 2b+1) gather both kv halves
        nc.gpsimd.collective_compute(
            "AllGather",
            mybir.AluOpType.bypass,
            replica_groups=[[0, 1], [2, 3], [4, 5], [6, 7]],
            ins=[kv_in.opt()],
            outs=[kv_out.opt()],
        )

        # load back full kT [128, 2048] and v [128 sk, 1024] tiles
        for t in range(8):
            for r in range(2):
                nc.sync.dma_start(
                    k_sb[t][:, SKH * r:SKH * (r + 1)],
                    kv_out[2 * SKH * r + 128 * t:
                           2 * SKH * r + 128 * (t + 1), :])
        for s in range(NKC):
            r, sl = s // 8, s % 8
            nc.sync.dma_start(
                v_sb[s][:, :],
                kv_out[2 * SKH * r + SKH + 128 * sl:
                       2 * SKH * r + SKH + 128 * (sl + 1), :])

        stageA2.close()
        stageA.close()   # free xT/cxT/weights SBUF + psum pools

        # ---------------- Phase B: attention per head-pair ----------------
        stageB = ExitStack()
        work = stageB.enter_context(tc.tile_pool(name="work", bufs=3))
        norm = stageB.enter_context(tc.tile_pool(name="norm", bufs=1))
        psS = stageB.enter_context(
            tc.tile_pool(name="psS", bufs=3, space="PSUM"))
        psAcc = stageB.enter_context(
            tc.tile_pool(name="psAcc", bufs=2, space="PSUM"))
        psSum = stageB.enter_context(
            tc.tile_pool(name="psSum", bufs=2, space="PSUM"))

        for hp in range(8):
            acc = psAcc.tile([128, SQH], f32, tag="apsum", name="apsum")
            sums = work.tile([128, 2 * SQH], f16, tag="sums", name="sums",
                             bufs=2)
            for c in range(NKC):
                pt = work.tile([128, 2 * SQH], f16, tag="p", name="ptile")
                for a in range(2):
                    sc = psS.tile([128, SQH], f32, tag="scores",
                                  name="scores")
                    nc.tensor.matmul(
                        sc[:, :],
                        lhsT=k_sb[hp][64 * a:64 * (a + 1),
                                      128 * c:128 * (c + 1)],
                        rhs=q_sb[hp][64 * a:64 * (a + 1), :],
                        tile_position=(64 * a, 0),
                        start=True, stop=True,
                    )
                    nc.scalar.activation(
                        pt[:, SQH * a:SQH * (a + 1)], sc[:, :],
                        AF.Exp, bias=mk_sb[:, c:c + 1], scale=0.125,
                    )
                if c == 0:
                    nc.vector.tensor_copy(sums[:, :], pt[:, :])
                else:
                    nc.vector.tensor_add(sums[:, :], sums[:, :], pt[:, :])
                for a in range(2):
                    nc.tensor.matmul(
                        acc[64 * a:64 * (a + 1), :],
                        lhsT=v_sb[c][:, 64 * (2 * hp + a):
                                     64 * (2 * hp + a + 1)],
                        rhs=pt[:, SQH * a:SQH * (a + 1)],
                        tile_position=(0, 64 * a),
                        start=(c == 0), stop=(c == NKC - 1),
                    )
            # normalization: R = 1 / colsum(exp) broadcast to partitions
            r_bc = norm.tile([128, 2 * SQH], f32, tag="rbc", name="rbc")
            for j in range(2):
                sp = psSum.tile([128, 512], f32, tag="sum_ps", name="sum_ps")
                nc.tensor.matmul(
                    sp[:, :], lhsT=om_sb[:, :],
                    rhs=sums[:, 512 * j:512 * (j + 1)],
                    start=True, stop=True,
                )
                nc.vector.reciprocal_approx_fast(
                    r_bc[:, 512 * j:512 * (j + 1)], sp[:, :])
            for a in range(2):
                nc.vector.tensor_mul(
                    a_sb[hp][64 * a:64 * (a + 1), :],
                    acc[64 * a:64 * (a + 1), :],
                    r_bc[64 * a:64 * (a + 1), SQH * a:SQH * (a + 1)],
                )
        stageB.close()

        # ---------------- Phase C: c_proj, natural [sq, d] layout ---------
        stageC = ExitStack()
        cpool = stageC.enter_context(tc.tile_pool(name="stageC", bufs=2))
        psO = stageC.enter_context(
            tc.tile_pool(name="psO", bufs=2, space="PSUM"))
        for sqc in range(4):
            ot = cpool.tile([128, D], f16, tag="outT", name="ot")
            for dc in range(2):
                ps = psO.tile([128, 512], f32, tag="ops", name="ops")
                for hp in range(8):
                    nc.tensor.matmul(
                        ps[:, :],
                        lhsT=a_sb[hp][:, 128 * sqc:128 * (sqc + 1)],
                        rhs=wp_sb[2 * hp + dc][:, :],
                        start=(hp == 0), stop=(hp == 7),
                    )
                nc.vector.tensor_add(
                    ot[:, 512 * dc:512 * (dc + 1)], ps[:, :],
                    bp_sb[:, 512 * dc:512 * (dc + 1)])
            nc.sync.dma_start(out_d[128 * sqc:128 * (sqc + 1), :], ot[:, :])
        stageC.close()

    nc.compile()
    return nc


class _Runner:
    """Cached jit(shard_map(bass_exec)) with device-resident weights."""

    def __init__(self, nc):
        import jax
        import concourse.mybir as mybir
        from concourse import bass2jax
        from jax.sharding import Mesh, NamedSharding, PartitionSpec
        from jax.experimental.shard_map import shard_map

        bass2jax.install_neuronx_cc_hook()
        self.jax = jax
        self.nc = nc

        partition_name = (nc.partition_id_tensor.name
                          if nc.partition_id_tensor else None)
        in_names, out_names, out_avals = [], [], []
        for alloc in nc.m.functions[0].allocations:
            if not isinstance(alloc, mybir.MemoryLocationSet):
                continue
            name = alloc.memorylocations[0].name
            if alloc.kind == "ExternalInput":
                if name != partition_name:
                    in_names.append(name)
            elif alloc.kind == "ExternalOutput":
                out_names.append(name)
                out_avals.append(jax.core.ShapedArray(
                    tuple(alloc.tensor_shape), mybir.dt.np(alloc.dtype)))
        self.in_names = in_names
        self.out_names = out_names
        self.out_avals = out_avals
        n_params = len(in_names)
        bind_names = tuple(in_names + out_names +
                           ([partition_name] if partition_name else []))

        devices = jax.devices()[:NCORES]
        self.mesh = Mesh(np.asarray(devices), ("core",))
        self.shard = NamedSharding(self.mesh, PartitionSpec("core"))

        def _body(*args):
            operands = list(args)
            if partition_name is not None:
                operands.append(bass2jax.partition_id_tensor())
            outs = bass2jax._bass_exec_p.bind(
                *operands,
                out_avals=tuple(out_avals),
                in_names=bind_names,
                out_names=tuple(out_names),
                lowering_input_output_aliases=(),
                sim_require_finite=True,
                sim_require_nnan=True,
                nc=nc,
            )
            return tuple(outs)

        donate = tuple(range(n_params, n_params + len(out_names)))
        self.sharded = jax.jit(
            shard_map(_body, mesh=self.mesh,
                      in_specs=(PartitionSpec("core"),) * (n_params +
                                                           len(out_names)),
                      out_specs=(PartitionSpec("core"),) * len(out_names),
                      check_rep=False),
            donate_argnums=donate, keep_unused=True)

        import jax.numpy as jnp
        zero_shards = [NamedSharding(self.mesh, PartitionSpec("core"))
                       for _ in out_avals]

        def _mk_zeros():
            return tuple(
                jnp.zeros((NCORES * a.shape[0],) + tuple(a.shape[1:]),
                          a.dtype)
                for a in out_avals)

        self.mk_zeros = jax.jit(_mk_zeros, out_shardings=tuple(zero_shards))

    def put(self, arr):
        """Host concat array -> device-resident sharded jax array."""
        return self.jax.device_put(arr, self.shard)

    def run(self, operands):
        """operands: dict name -> array (jax device or numpy concat)."""
        zeros = self.mk_zeros()
        args = [operands[n] for n in self.in_names] + list(zeros)
        outs = self.sharded(*args)
        return outs


def _checksum(arr):
    """Full-integrity checksum (uint32-view sum) — cheap vs the transfer."""
    a = np.ascontiguousarray(arr)
    return (a.shape, a.dtype.str,
            int(a.view(np.uint32).sum(dtype=np.uint64)))


def _prep_weights(runner, Wq, bq, Wkv, bkv, Wp, bp):
    """Device-resident per-core weight shards (identical on all cores)."""
    f16, f32 = np.float16, np.float32

    def rep(a):   # replicate per core along axis0 for shard_map concat
        return np.ascontiguousarray(
            np.broadcast_to(a, (NCORES,) + a.shape)).reshape(
                (NCORES * a.shape[0],) + a.shape[1:])

    wq = Wq.astype(f16)
    wk = Wkv[:, :D].astype(f16)
    wv = Wkv[:, D:].astype(f16)
    wp = Wp.astype(f16)
    bq_t = np.ascontiguousarray(bq.astype(f32).reshape(8, 128).T)
    bk_t = np.ascontiguousarray(bkv[:D].astype(f32).reshape(8, 128).T)
    bv_bc = np.ascontiguousarray(
        np.broadcast_to(bkv[D:].astype(f32), (128, D)))
    bp_bc = np.ascontiguousarray(
        np.broadcast_to(bp.astype(f32), (128, D)))
    ident = np.eye(128, dtype=f16)
    ones_m = np.ones((128, 128), dtype=f16)
    host = {
        "wq": rep(wq), "wk": rep(wk), "wv": rep(wv), "wp": rep(wp),
        "bq_t": rep(bq_t), "bk_t": rep(bk_t), "bv_bc": rep(bv_bc),
        "bp_bc": rep(bp_bc), "ident": rep(ident), "ones_m": rep(ones_m),
    }
    return {k: runner.put(v) for k, v in host.items()}


def _prep_acts(runner, x, ctx, attention_mask):
    """Per-call activations: natural layout f16, zero-copy reshapes."""
    f16, f32 = np.float16, np.float32
    x16 = np.asarray(x, dtype=f16)                       # [4,1024,1024]
    cx16 = np.asarray(ctx, dtype=f16)                    # [4,2048,1024]
    m = np.asarray(attention_mask, dtype=f32).reshape(B, SK)
    mask_t = np.ascontiguousarray(
        m.reshape(B, NKC, 128).transpose(0, 2, 1))       # [4,128,16]
    concat_mask = np.ascontiguousarray(
        mask_t[[0, 0, 1, 1, 2, 2, 3, 3]]).reshape(NCORES * 128, NKC)
    return {"x": x16.reshape(NCORES * SQH, D),
            "cx": cx16.reshape(NCORES * (SK // 2), D),
            "mask_t": concat_mask}


def kernel(x, ctx, attention_mask, Wq, bq, Wkv, bkv, Wp, bp, **_ignored):
    x = np.asarray(x); ctx = np.asarray(ctx)
    attention_mask = np.asarray(attention_mask)
    Wq = np.asarray(Wq); bq = np.asarray(bq)
    Wkv = np.asarray(Wkv); bkv = np.asarray(bkv)
    Wp = np.asarray(Wp); bp = np.asarray(bp)

    if "nc" not in _CACHE:
        _CACHE["nc"] = _build_program()
    nc = _CACHE["nc"]
    if "runner" not in _CACHE:
        _CACHE["runner"] = _Runner(nc)
    runner = _CACHE["runner"]

    wkey = tuple(_checksum(a) for a in (Wq, bq, Wkv, bkv, Wp, bp))
    if _CACHE.get("wkey") != wkey:
        _CACHE["weights"] = _prep_weights(runner, Wq, bq, Wkv, bkv, Wp, bp)
        _CACHE["wkey"] = wkey

    operands = dict(_CACHE["weights"])
    operands.update(_prep_acts(runner, x, ctx, attention_mask))
    outs = runner.run(operands)
    out16 = np.asarray(outs[0])                          # [8*512, 1024] f16
    return out16.reshape(B, SQ, D).astype(np.float32)
